# revision 1
# baseline (speedup 1.0000x reference)
"""Trainium2 Bass kernel for a 2-layer GCN (GCNConv -> ReLU -> GCNConv -> sigmoid head).

Strategy (8 NeuronCores):
  - Node sharding: core c owns nodes [c*12500, (c+1)*12500), padded to 12544 = 98*128.
  - Edges are assigned to the core that owns their dst node (so segment-sums are local).
  - Using GCN algebra:  agg[i] = dis[i] * sum_{e: dst=i} (dis*h)[src_e] + (1/deg_i)*h[i] + b
    so the per-edge norm multiply disappears; rows are pre-scaled by dis once per node.
  - Per layer: local dense matmul h = x@W, scale by dis, AllGather a bf16 feature table
    (rows padded to 128 cols = 256B so dma_gather's elem-size constraint holds), then for
    each (src-chunk, dst-tile) cell gather the needed source rows with dma_gather (int16
    chunk-relative indices) and segment-sum via one-hot matmuls on the tensor engine.
"""

import numpy as np
import ml_dtypes

P = 128


class Cfg:
    def __init__(self, n_nodes, n_loc_real, nt, in_c, hid, nchunk, group_tiles=8):
        self.C = 8
        self.N = n_nodes
        self.NLOC_REAL = n_loc_real           # real nodes per core
        self.NT = nt                          # node tiles per core
        self.NLOC = nt * P                    # padded nodes per core
        self.NTAB = self.C * self.NLOC        # global table rows
        self.IN_C = in_c
        self.HID = hid
        self.TABW = 128                       # table row width (bf16) -> 256B rows
        self.NCHUNK = nchunk
        self.CR = self.NTAB // nchunk         # chunk rows (must be < 32768)
        assert self.CR * nchunk == self.NTAB and self.CR < 32768
        # tile groups for gathers: (tile_start, ntiles)
        self.groups = []
        t = 0
        while t < nt:
            g = min(group_tiles, nt - t)
            self.groups.append((t, g))
            t += g


def full_cfg():
    return Cfg(n_nodes=100000, n_loc_real=12500, nt=98, in_c=128, hid=64, nchunk=4,
               group_tiles=1)


def _prep(cfg, x, edge_index, W1, b1, W2, b2, Wl, bl):
    """Host-side sharding/partitioning. Returns (in_maps, B)."""
    C, NT, NLOC, NLOC_REAL = cfg.C, cfg.NT, cfg.NLOC, cfg.NLOC_REAL
    src = np.asarray(edge_index[0], dtype=np.int64)
    dst = np.asarray(edge_index[1], dtype=np.int64)
    core = dst // NLOC_REAL
    dst_local = (dst - core * NLOC_REAL).astype(np.int64)
    # global table row id (cores are padded to NLOC rows each)
    src_adj = (src // NLOC_REAL) * NLOC + (src % NLOC_REAL)
    chunk = src_adj // cfg.CR
    tile = dst_local // P

    ncell = cfg.NCHUNK * NT
    cell = (core * ncell + chunk * NT + tile).astype(np.int64)
    counts = np.bincount(cell, minlength=C * ncell)
    B = max(1, int(np.ceil(counts.max() / P)))
    S = B * P                                   # slots per cell
    NSLOT = ncell * S                           # per core
    NBLK = ncell * B

    order = np.argsort(cell, kind="stable")
    cell_s = cell[order]
    cell_start = np.zeros(C * ncell + 1, dtype=np.int64)
    np.cumsum(counts, out=cell_start[1:])
    rank = np.arange(len(src)) - cell_start[cell_s]
    slot_global = (cell_s % ncell) * S + rank + (cell_s // ncell) * NSLOT

    idx16 = np.zeros(C * NSLOT, dtype=np.int16)
    dstrel = np.full(C * NSLOT, -1.0, dtype=np.float32)
    idx16[slot_global] = (src_adj[order] - chunk[order] * cfg.CR).astype(np.int16)
    dstrel[slot_global] = (dst_local[order] - tile[order] * P).astype(np.float32)

    in_maps = []
    for c in range(C):
        m = {}
        xl = np.zeros((P, NLOC), dtype=np.float32)
        xl[:, :NLOC_REAL] = np.asarray(x[c * NLOC_REAL:(c + 1) * NLOC_REAL], np.float32).T
        m["xT"] = np.ascontiguousarray(xl)

        cnt = np.bincount(dst_local[core == c], minlength=NLOC).astype(np.int64)
        rowptr = np.concatenate([[0], np.cumsum(cnt)])
        m["rp0"] = np.ascontiguousarray(rowptr[:-1].reshape(NT, P).T.astype(np.int32))
        m["rp1"] = np.ascontiguousarray(rowptr[1:].reshape(NT, P).T.astype(np.int32))

        # gather indices, wrapped per call: call order = (chunk, group); within a call
        # logical index i -> partition i%16, col i//16; replicated to 8 groups of 16 parts
        base = c * NSLOT
        cols = []
        for ch in range(cfg.NCHUNK):
            for (t0, g) in cfg.groups:
                seg = idx16[base + (ch * NT + t0) * S: base + (ch * NT + t0 + g) * S]
                w = seg.reshape(-1, 16).T
                cols.append(np.tile(w, (8, 1)))
        m["gidx"] = np.ascontiguousarray(np.concatenate(cols, axis=1))

        dr = dstrel[base: base + NSLOT].reshape(NBLK, P).T
        m["dstrel"] = np.ascontiguousarray(dr.astype(np.float32))

        m["identm"] = np.eye(P, dtype=np.float32)
        m["iota"] = np.ascontiguousarray(
            np.tile(np.arange(P, dtype=np.float32), (P, 1)).astype(ml_dtypes.bfloat16))
        m["W1"] = np.asarray(W1, np.float32)
        m["W2"] = np.asarray(W2, np.float32)
        m["b1b"] = np.ascontiguousarray(np.tile(np.asarray(b1, np.float32)[None, :], (P, 1)))
        m["b2b"] = np.ascontiguousarray(np.tile(np.asarray(b2, np.float32)[None, :], (P, 1)))
        m["Wlb"] = np.ascontiguousarray(np.tile(np.asarray(Wl, np.float32)[:, 0][None, :], (P, 1)))
        in_maps.append(m)
    return in_maps, B


def _program(cfg, B, bl_value, no_gather=False, linearize=False):
    from concourse import bass, bacc, mybir
    import concourse.tile as tile

    f32 = mybir.dt.float32
    bf16 = mybir.dt.bfloat16
    i32 = mybir.dt.int32
    i16 = mybir.dt.int16
    AF = mybir.ActivationFunctionType
    OP = mybir.AluOpType

    NT, NLOC, HID, TABW, CR = cfg.NT, cfg.NLOC, cfg.HID, cfg.TABW, cfg.CR
    S = B * P
    NBLK = cfg.NCHUNK * NT * B
    NCOL = cfg.NCHUNK * NT * S // 16
    groups = cfg.groups
    rg = [list(range(cfg.C))]

    nc = bacc.Bacc("TRN2", target_bir_lowering=False, debug=False,
                   num_devices=cfg.C)
    xT_d = nc.dram_tensor("xT", [P, NLOC], f32, kind="ExternalInput")
    rp0_d = nc.dram_tensor("rp0", [P, NT], i32, kind="ExternalInput")
    rp1_d = nc.dram_tensor("rp1", [P, NT], i32, kind="ExternalInput")
    gidx_d = nc.dram_tensor("gidx", [P, NCOL], i16, kind="ExternalInput")
    dstrel_d = nc.dram_tensor("dstrel", [P, NBLK], f32, kind="ExternalInput")
    iota_d = nc.dram_tensor("iota", [P, P], bf16, kind="ExternalInput")
    identm_d = nc.dram_tensor("identm", [P, P], f32, kind="ExternalInput")
    W1_d = nc.dram_tensor("W1", [cfg.IN_C, HID], f32, kind="ExternalInput")
    W2_d = nc.dram_tensor("W2", [HID, HID], f32, kind="ExternalInput")
    b1b_d = nc.dram_tensor("b1b", [P, HID], f32, kind="ExternalInput")
    b2b_d = nc.dram_tensor("b2b", [P, HID], f32, kind="ExternalInput")
    Wlb_d = nc.dram_tensor("Wlb", [P, HID], f32, kind="ExternalInput")
    out_d = nc.dram_tensor("out", [NT, P], f32, kind="ExternalOutput")

    hloc = [nc.dram_tensor(f"h{l}loc", [NLOC, TABW], bf16) for l in (1, 2)]
    tabs = [nc.dram_tensor(f"tab{l}", [cfg.NTAB, TABW], bf16, addr_space="Shared")
            for l in (1, 2)]

    with tile.TileContext(nc, linearize=linearize) as tc:
        from contextlib import ExitStack
        with ExitStack() as ctx:
            const = ctx.enter_context(tc.tile_pool(name="const", bufs=1))
            persist = ctx.enter_context(tc.tile_pool(name="persist", bufs=1))
            tmp = ctx.enter_context(tc.tile_pool(name="tmp", bufs=3))
            psum = ctx.enter_context(tc.tile_pool(name="psum", bufs=2, space="PSUM"))

            nreg = {}
            ident = const.tile([P, P], f32, tag="ident")
            nc.sync.dma_start(out=ident[:], in_=identm_d[:, :])
            iota_t = const.tile([P, P], bf16, tag="iota")
            nc.sync.dma_start(out=iota_t[:], in_=iota_d[:, :])
            W1_t = const.tile([cfg.IN_C, HID], f32, tag="W1")
            nc.sync.dma_start(out=W1_t[:], in_=W1_d[:, :])
            W2_t = const.tile([HID, HID], f32, tag="W2")
            nc.sync.dma_start(out=W2_t[:], in_=W2_d[:, :])
            b1_t = const.tile([P, HID], f32, tag="b1")
            nc.sync.dma_start(out=b1_t[:], in_=b1b_d[:, :])
            b2_t = const.tile([P, HID], f32, tag="b2")
            nc.sync.dma_start(out=b2_t[:], in_=b2b_d[:, :])
            Wl_t = const.tile([P, HID], f32, tag="Wl")
            nc.sync.dma_start(out=Wl_t[:], in_=Wlb_d[:, :])
            bl_t = const.tile([P, 1], f32, tag="bl")
            nc.vector.memset(bl_t[:], float(bl_value))
            dstrel_t = const.tile([P, NBLK], f32, tag="dstrel")
            nc.sync.dma_start(out=dstrel_t[:], in_=dstrel_d[:, :])

            # deg -> dis = sqrt(1/deg), selfw = 1/deg
            rp0_t = const.tile([P, NT], i32, tag="rp0")
            nc.sync.dma_start(out=rp0_t[:], in_=rp0_d[:, :])
            rp1_t = const.tile([P, NT], i32, tag="rp1")
            nc.sync.dma_start(out=rp1_t[:], in_=rp1_d[:, :])
            degi = const.tile([P, NT], i32, tag="degi")
            nc.vector.tensor_tensor(out=degi[:], in0=rp1_t[:], in1=rp0_t[:], op=OP.subtract)
            degf = const.tile([P, NT], f32, tag="degf")
            nc.vector.tensor_copy(degf[:], degi[:])
            deg = const.tile([P, NT], f32, tag="deg")
            nc.vector.tensor_scalar(out=deg[:], in0=degf[:], scalar1=1.0, scalar2=None,
                                    op0=OP.add)
            selfw = const.tile([P, NT], f32, tag="selfw")
            nc.vector.reciprocal(out=selfw[:], in_=deg[:])
            dis = const.tile([P, NT], f32, tag="dis")
            nc.scalar.activation(out=dis[:], in_=selfw[:], func=AF.Sqrt)

            h_sb = persist.tile([P, NT * HID], f32, tag="h_sb")
            hp_sb = persist.tile([P, NT * TABW], bf16, tag="hp_sb")
            acc_sb = persist.tile([P, NT * HID], f32, tag="acc_sb")
            zT_sb = persist.tile([HID, NT * P], f32, tag="zT_sb")
            y_sb = persist.tile([P, NT], f32, tag="y_sb")
            # zero the bf16 table pad columns once (cols HID..TABW of each tile row)
            nc.vector.memset(hp_sb[:], 0.0)

            def layer_A(l, xT_t):
                """h = in @ W; h' = dis*h (bf16, into hp_sb); DMA h' to hloc[l-1]."""
                W_t = W1_t if l == 1 else W2_t
                for t in range(NT):
                    ps = psum.tile([P, HID], f32, tag="psA")
                    if l == 1:
                        lhsT = xT_t[:, t * P:(t + 1) * P]
                    else:
                        lhsT = zT_sb[:, t * P:(t + 1) * P]
                    nc.tensor.matmul(out=ps[:], lhsT=lhsT, rhs=W_t[:], start=True, stop=True)
                    nc.scalar.copy(out=h_sb[:, t * HID:(t + 1) * HID], in_=ps[:])
                    nc.scalar.activation(out=hp_sb[:, t * TABW:t * TABW + HID], in_=ps[:],
                                         func=AF.Copy, scale=dis[:, t:t + 1])
                for t in range(NT):
                    nc.sync.dma_start(out=hloc[l - 1][t * P:(t + 1) * P, :],
                                      in_=hp_sb[:, t * TABW:(t + 1) * TABW])

            def layer_agg(l):
                """AllGather table, gather+segment-sum into acc_sb."""
                nc.gpsimd.collective_compute(
                    "AllGather", mybir.AluOpType.bypass, replica_groups=rg,
                    ins=[hloc[l - 1][:, :]], outs=[tabs[l - 1][:, :]])
                with tc.tile_pool(name=f"gath{l}", bufs=2) as gp, \
                     tc.tile_pool(name=f"gidx{l}", bufs=3) as gip:
                    col = 0
                    blk = 0
                    for ch in range(cfg.NCHUNK):
                        for (t0, g) in groups:
                            nI = g * S
                            gi = gip.tile([P, nI // 16], i16, tag="gi")
                            nc.sync.dma_start(out=gi[:], in_=gidx_d[:, col:col + nI // 16])
                            col += nI // 16
                            gf = gp.tile([P, g * B, TABW], bf16, tag="gf")
                            if no_gather:
                                nc.vector.memset(gf[:], 0.0)
                            else:
                                nc.gpsimd.dma_gather(
                                    out_ap=gf[:], in_ap=tabs[l - 1][ch * CR:(ch + 1) * CR, :],
                                    idxs_ap=gi[:], num_idxs=nI,
                                    num_idxs_reg=nreg.setdefault(g, nc.gpsimd.to_reg(g * S)),
                                    elem_size=TABW)
                            ps = psum.tile([P, g * HID], f32, tag="psC")
                            for ti in range(g):
                                for b in range(B):
                                    oh = tmp.tile([P, P], bf16, tag="oh")
                                    nc.vector.tensor_scalar(
                                        out=oh[:], in0=iota_t[:],
                                        scalar1=dstrel_t[:, blk:blk + 1], scalar2=None,
                                        op0=OP.is_equal)
                                    nc.tensor.matmul(
                                        out=ps[:, ti * HID:(ti + 1) * HID],
                                        lhsT=oh[:],
                                        rhs=gf[:, ti * B + b, 0:HID],
                                        start=(b == 0), stop=(b == B - 1))
                                    blk += 1
                            dstslice = acc_sb[:, t0 * HID:(t0 + g) * HID]
                            if ch == 0:
                                nc.scalar.copy(out=dstslice, in_=ps[:])
                            else:
                                nc.vector.tensor_tensor(out=dstslice, in0=dstslice,
                                                        in1=ps[:], op=OP.add)

            def layer_post(l):
                """agg = dis*s + selfw*h + b; l1: relu+transpose into zT; l2: head."""
                b_t = b1_t if l == 1 else b2_t
                for t in range(NT):
                    t1 = tmp.tile([P, HID], f32, tag="t1")
                    nc.scalar.activation(out=t1[:], in_=acc_sb[:, t * HID:(t + 1) * HID],
                                         func=AF.Copy, scale=dis[:, t:t + 1])
                    t2 = tmp.tile([P, HID], f32, tag="t2")
                    nc.scalar.activation(out=t2[:], in_=h_sb[:, t * HID:(t + 1) * HID],
                                         func=AF.Copy, scale=selfw[:, t:t + 1])
                    nc.vector.tensor_tensor(out=t1[:], in0=t1[:], in1=t2[:], op=OP.add)
                    nc.vector.tensor_tensor(out=t1[:], in0=t1[:], in1=b_t[:], op=OP.add)
                    if l == 1:
                        z = tmp.tile([P, HID], f32, tag="z")
                        nc.scalar.activation(out=z[:], in_=t1[:], func=AF.Relu)
                        psE = psum.tile([HID, P], f32, tag="psE")
                        nc.tensor.transpose(out=psE[:], in_=z[:], identity=ident[:])
                        nc.scalar.copy(
                            out=zT_sb[:, t * P:(t + 1) * P], in_=psE[:])
                    else:
                        m = tmp.tile([P, HID], f32, tag="m")
                        nc.vector.tensor_tensor(out=m[:], in0=t1[:], in1=Wl_t[:], op=OP.mult)
                        r = tmp.tile([P, 1], f32, tag="r")
                        nc.vector.tensor_reduce(out=r[:], in_=m[:],
                                                axis=mybir.AxisListType.X, op=OP.add)
                        nc.scalar.activation(out=y_sb[:, t:t + 1], in_=r[:],
                                             func=AF.Sigmoid, bias=bl_t[:, 0:1])

            with tc.tile_pool(name="xt", bufs=1) as xtp:
                xT_t = xtp.tile([P, NLOC], f32, tag="xT")
                nc.sync.dma_start(out=xT_t[:], in_=xT_d[:, :])
                layer_A(1, xT_t)
            layer_agg(1)
            layer_post(1)
            layer_A(2, None)
            layer_agg(2)
            layer_post(2)

            psG = psum.tile([NT, P], f32, tag="psG")
            nc.tensor.matmul(out=psG[:], lhsT=y_sb[:, :NT], rhs=ident[:],
                             start=True, stop=True, is_transpose=True)
            og = tmp.tile([NT, P], f32, tag="og")
            nc.scalar.copy(out=og[:], in_=psG[:])
            nc.sync.dma_start(out=out_d[:, :], in_=og[:])
    nc.compile()
    return nc


def kernel(x, edge_index, W1, b1, W2, b2, Wl, bl):
    from concourse.bass_utils import run_bass_kernel_spmd
    cfg = full_cfg()
    in_maps, B = _prep(cfg, x, edge_index, W1, b1, W2, b2, Wl, bl)
    nc = _program(cfg, B, float(np.asarray(bl).reshape(-1)[0]))
    res = run_bass_kernel_spmd(nc, in_maps, list(range(cfg.C)))
    outs = []
    for c in range(cfg.C):
        o = np.asarray(res.results[c]["out"], dtype=np.float32).reshape(cfg.NLOC)
        outs.append(o[:cfg.NLOC_REAL])
    return np.concatenate(outs).reshape(cfg.N, 1).astype(np.float32)



# revision 2
# speedup vs baseline: 1.2581x; 1.2581x over previous
"""Trainium2 Bass kernel v4 for a 2-layer GCN (GCNConv -> ReLU -> GCNConv -> sigmoid).

v4 over v3: quarter-split AllGathers (4 independent collectives per layer over
separate hlocq/tabq tensors) fired as soon as each quarter of layer-A finishes,
plus layer-2's A phase interleaved into the layer-1 scatter loop, so collective
latency and the second layer's h-compute hide under the gather phase.

Table layout: table row for (core k, local node i) lives in quarter q = i//QL,
at row k*QL + i%QL of tabq[q] (QL = NLOC/4 = 3136). A chunk for int16 gather
addressing = one quarter tensor (25088 rows < 32768).
"""

import numpy as np
import ml_dtypes

P = 128
NCORES = 8
N_NODES = 100000
NLOC_REAL = 12500
NT = 98
NLOC = NT * P                  # 12544
QL = NLOC // 4                 # 3136 local rows per quarter
CR = NCORES * QL               # 25088 rows per quarter table (< 32768)
NCHUNK = 4
IN_C = 128
HID = 64
TABW = 128                     # table row width (bf16) -> 256B dma_gather elem
BMAX = 8


def _prep(x, edge_index, W1, b1, W2, b2, Wl, bl):
    src = np.asarray(edge_index[0], dtype=np.int64)
    dst = np.asarray(edge_index[1], dtype=np.int64)
    core = dst // NLOC_REAL
    dst_local = dst - core * NLOC_REAL
    sk = src // NLOC_REAL                     # source core
    si = src - sk * NLOC_REAL                 # source local index
    chunk = si // QL                          # quarter
    src_rel = sk * QL + (si - chunk * QL)     # row within tabq[chunk]
    tile = dst_local // P

    ncell = NT * NCHUNK
    cell = core * ncell + tile * NCHUNK + chunk
    order = np.argsort(cell, kind="stable")
    counts = np.bincount(cell, minlength=NCORES * ncell)
    src_s = src_rel[order].astype(np.int16)
    dstl_s = (dst_local[order] - tile[order] * P).astype(np.float32)
    cell_start = np.zeros(NCORES * ncell + 1, dtype=np.int64)
    np.cumsum(counts, out=cell_start[1:])

    cnt_ctc = counts.reshape(NCORES, NT, NCHUNK)
    r16_tc = np.maximum(16, -(-cnt_ctc.max(axis=0) // 16) * 16)
    nb_tc = -(-r16_tc // P)
    assert nb_tc.max() <= BMAX, f"cell needs {nb_tc.max()} blocks (> {BMAX})"

    tiles = []
    nbt = []
    blk0 = []
    col_off = 0
    blk_off = 0
    for t in range(NT):
        calls = []
        blk0.append(blk_off)
        for ch in range(NCHUNK):
            nb = int(nb_tc[t, ch])
            rows16 = int(r16_tc[t, ch])
            calls.append((ch, nb, col_off, rows16))
            col_off += rows16 // 16
            blk_off += nb
        tiles.append(calls)
        nbt.append(int(nb_tc[t].sum()))
    meta = {"tiles": tiles, "nbt": nbt, "blk0": blk0, "nblk": blk_off,
            "gcols": col_off, "nbt_max": max(nbt)}

    in_maps = []
    for c in range(NCORES):
        m = {}
        xl = np.zeros((P, NLOC), dtype=np.float32)
        xl[:, :NLOC_REAL] = np.asarray(
            x[c * NLOC_REAL:(c + 1) * NLOC_REAL], np.float32).T
        m["xT"] = np.ascontiguousarray(xl)

        deg = np.bincount(dst_local[core == c], minlength=NLOC).astype(np.float64)
        deg += 1.0
        m["dis"] = np.ascontiguousarray(
            (1.0 / np.sqrt(deg)).astype(np.float32).reshape(NT, P).T)

        gidx = np.zeros((P, meta["gcols"]), dtype=np.int16)
        dstrel = np.full((P, meta["nblk"]), -1.0, dtype=np.float32)
        for t in range(NT):
            for (ch, nb, coff, rows16) in tiles[t]:
                cid = c * ncell + t * NCHUNK + ch
                lo, hi = cell_start[cid], cell_start[cid + 1]
                cnt = int(hi - lo)
                idx16 = np.zeros(rows16, dtype=np.int16)
                drel = np.full(nb * P, -1.0, dtype=np.float32)
                idx16[:cnt] = src_s[lo:hi]
                drel[:cnt] = dstl_s[lo:hi]
                w = idx16.reshape(-1, 16).T
                gidx[:, coff:coff + rows16 // 16] = np.tile(w, (8, 1))
                b0 = blk0[t] + sum(nb2 for (_, nb2, _co2, _r2) in tiles[t]
                                   if _co2 < coff)
                dstrel[:, b0:b0 + nb] = drel.reshape(nb, P).T
        m["gidx"] = np.ascontiguousarray(gidx)
        m["dstrel"] = np.ascontiguousarray(dstrel.astype(ml_dtypes.bfloat16))

        m["identm"] = np.eye(P, dtype=np.float32)
        m["iota"] = np.ascontiguousarray(
            np.tile(np.arange(P, dtype=np.float32), (P, 1)).astype(
                ml_dtypes.bfloat16))
        m["W1"] = np.asarray(W1, np.float32)
        m["W2"] = np.asarray(W2, np.float32)
        m["b1b"] = np.ascontiguousarray(
            np.tile(np.asarray(b1, np.float32)[None, :], (P, 1)))
        m["b2b"] = np.ascontiguousarray(
            np.tile(np.asarray(b2, np.float32)[None, :], (P, 1)))
        m["Wlb"] = np.ascontiguousarray(
            np.tile(np.asarray(Wl, np.float32)[:, 0][None, :], (P, 1)))
        in_maps.append(m)
    return in_maps, meta


def _program(meta, bl_value, linearize=False):
    from concourse import bass, bacc, mybir
    import concourse.tile as tile

    f32 = mybir.dt.float32
    bf16 = mybir.dt.bfloat16
    i16 = mybir.dt.int16
    AF = mybir.ActivationFunctionType
    OP = mybir.AluOpType

    tiles, nbt, blk0 = meta["tiles"], meta["nbt"], meta["blk0"]
    NBLK, GCOLS, NBT_MAX = meta["nblk"], meta["gcols"], meta["nbt_max"]

    nc = bacc.Bacc("TRN2", target_bir_lowering=False, debug=False,
                   num_devices=NCORES, num_swdge_queues=4)
    xT_d = nc.dram_tensor("xT", [P, NLOC], f32, kind="ExternalInput")
    dis_d = nc.dram_tensor("dis", [P, NT], f32, kind="ExternalInput")
    gidx_d = nc.dram_tensor("gidx", [P, GCOLS], i16, kind="ExternalInput")
    dstrel_d = nc.dram_tensor("dstrel", [P, NBLK], bf16, kind="ExternalInput")
    iota_d = nc.dram_tensor("iota", [P, P], bf16, kind="ExternalInput")
    identm_d = nc.dram_tensor("identm", [P, P], f32, kind="ExternalInput")
    W1_d = nc.dram_tensor("W1", [IN_C, HID], f32, kind="ExternalInput")
    W2_d = nc.dram_tensor("W2", [HID, HID], f32, kind="ExternalInput")
    b1b_d = nc.dram_tensor("b1b", [P, HID], f32, kind="ExternalInput")
    b2b_d = nc.dram_tensor("b2b", [P, HID], f32, kind="ExternalInput")
    Wlb_d = nc.dram_tensor("Wlb", [P, HID], f32, kind="ExternalInput")
    out_d = nc.dram_tensor("out", [NT, P], f32, kind="ExternalOutput")

    # per-layer, per-quarter local slices and gathered tables
    hlocq = [[nc.dram_tensor(f"h{l}q{q}", [QL, TABW], bf16) for q in range(4)]
             for l in (1, 2)]
    tabq = [[nc.dram_tensor(f"tab{l}q{q}", [CR, TABW], bf16,
                            addr_space="Shared") for q in range(4)]
            for l in (1, 2)]
    rg = [list(range(NCORES))]

    with tile.TileContext(nc, linearize=linearize) as tc:
        from contextlib import ExitStack
        with ExitStack() as ctx:
            const = ctx.enter_context(tc.tile_pool(name="const", bufs=1))
            persist = ctx.enter_context(tc.tile_pool(name="persist", bufs=1))
            tmp = ctx.enter_context(tc.tile_pool(name="tmp", bufs=6))
            psum = ctx.enter_context(tc.tile_pool(name="psum", bufs=3,
                                                  space="PSUM"))
            psumT = ctx.enter_context(tc.tile_pool(name="psumT", bufs=1,
                                                   space="PSUM"))

            ident = const.tile([P, P], f32, tag="ident")
            nc.sync.dma_start(out=ident[:], in_=identm_d[:, :])
            identb = const.tile([P, P], bf16, tag="identb")
            nc.vector.tensor_copy(identb[:], ident[:])
            iota_t = const.tile([P, P], bf16, tag="iota")
            nc.sync.dma_start(out=iota_t[:], in_=iota_d[:, :])
            W1_t = const.tile([IN_C, HID], f32, tag="W1")
            nc.sync.dma_start(out=W1_t[:], in_=W1_d[:, :])
            W2_t = const.tile([HID, HID], f32, tag="W2")
            nc.sync.dma_start(out=W2_t[:], in_=W2_d[:, :])
            b1_t = const.tile([P, HID], f32, tag="b1")
            nc.sync.dma_start(out=b1_t[:], in_=b1b_d[:, :])
            b2_t = const.tile([P, HID], f32, tag="b2")
            nc.sync.dma_start(out=b2_t[:], in_=b2b_d[:, :])
            Wl_t = const.tile([P, HID], f32, tag="Wl")
            nc.sync.dma_start(out=Wl_t[:], in_=Wlb_d[:, :])
            bl_t = const.tile([P, 1], f32, tag="bl")
            nc.vector.memset(bl_t[:], float(bl_value))
            dis_t = const.tile([P, NT], f32, tag="dis")
            nc.sync.dma_start(out=dis_t[:], in_=dis_d[:, :])
            dstrel_t = const.tile([P, NBLK], bf16, tag="dstrel")
            nc.sync.dma_start(out=dstrel_t[:], in_=dstrel_d[:, :])
            gidx_t = const.tile([P, GCOLS], i16, tag="gidx")
            nc.sync.dma_start(out=gidx_t[:], in_=gidx_d[:, :])

            hp_sb1 = persist.tile([P, NT * HID], bf16, tag="hp_sb1")
            hp_sb2 = persist.tile([P, NT * HID], bf16, tag="hp_sb2")
            hp_sb = [hp_sb1, hp_sb2]
            zT_sb = persist.tile([HID, NT * P], f32, tag="zT_sb")
            y_sb = persist.tile([P, NT], f32, tag="y_sb")

            nreg = {}

            def reg_for(n):
                if n not in nreg:
                    nreg[n] = nc.gpsimd.to_reg(n)
                return nreg[n]

            def tile_A(l, t, xT_t):
                """h tile t of layer l: matmul, scale to bf16, DMA to quarters."""
                W_t = W1_t if l == 1 else W2_t
                ps = psum.tile([P, HID], f32, tag="psA")
                if l == 1:
                    lhsT = xT_t[:, t * P:(t + 1) * P]
                else:
                    lhsT = zT_sb[:, t * P:(t + 1) * P]
                nc.tensor.matmul(out=ps[:], lhsT=lhsT, rhs=W_t[:],
                                 start=True, stop=True)
                hp = hp_sb[l - 1][:, t * HID:(t + 1) * HID]
                nc.scalar.activation(out=hp, in_=ps[:], func=AF.Copy,
                                     scale=dis_t[:, t:t + 1])
                r0, r1 = t * P, (t + 1) * P
                q0, q1 = r0 // QL, (r1 - 1) // QL
                for q in range(q0, q1 + 1):
                    a = max(r0, q * QL)
                    b = min(r1, (q + 1) * QL)
                    nc.sync.dma_start(
                        out=hlocq[l - 1][q][a - q * QL:b - q * QL, 0:HID],
                        in_=hp[a - r0:b - r0, :])

            def fire_AG(l, q):
                nc.gpsimd.collective_compute(
                    "AllGather", mybir.AluOpType.bypass, replica_groups=rg,
                    ins=[hlocq[l - 1][q][:, :]], outs=[tabq[l - 1][q][:, :]])

            # quarter q's last contributing tile: ceil((q+1)*QL / P) - 1
            q_last_tile = [-(-(q + 1) * QL // P) - 1 for q in range(4)]

            qn_state = [0]

            def tile_B(l, t, gp, ohp, xT_t=None):
                """gather+scatter+post for tile t of layer l; optionally emit
                layer-2 A for this tile right after the post (l==1)."""
                b_t = b1_t if l == 1 else b2_t
                nb_tot = nbt[t]
                oh = ohp.tile([P, NBT_MAX, P], bf16, tag="oh")
                nc.vector.tensor_tensor(
                    out=oh[:, 0:nb_tot, :],
                    in0=iota_t[:].unsqueeze(1).broadcast_to([P, nb_tot, P]),
                    in1=dstrel_t[:, blk0[t]:blk0[t] + nb_tot]
                        .unsqueeze(2).broadcast_to([P, nb_tot, P]),
                    op=OP.is_equal)
                ps = psum.tile([P, HID], f32, tag="psB")
                nc.tensor.matmul(
                    out=ps[:], lhsT=identb[:],
                    rhs=hp_sb[l - 1][:, t * HID:(t + 1) * HID],
                    start=True, stop=False)
                blk = 0
                for (ch, nb, coff, rows16) in tiles[t]:
                    gf = gp.tile([P, BMAX, TABW], bf16, tag="gf")
                    nc.gpsimd.dma_gather(
                        out_ap=gf[:, 0:nb, :],
                        in_ap=tabq[l - 1][ch][:, :],
                        idxs_ap=gidx_t[:, coff:coff + rows16 // 16],
                        num_idxs=rows16, num_idxs_reg=reg_for(rows16),
                        elem_size=TABW, queue_num=qn_state[0])
                    qn_state[0] = (qn_state[0] + 1) % 4
                    for b in range(nb):
                        nc.tensor.matmul(
                            out=ps[:],
                            lhsT=oh[:, blk + b, :],
                            rhs=gf[:, b, 0:HID],
                            start=False,
                            stop=(blk + b == nb_tot - 1))
                    blk += nb
                t1 = tmp.tile([P, HID], f32, tag="t1")
                nc.scalar.activation(out=t1[:], in_=ps[:], func=AF.Copy,
                                     scale=dis_t[:, t:t + 1])
                nc.vector.tensor_tensor(out=t1[:], in0=t1[:], in1=b_t[:],
                                        op=OP.add)
                if l == 1:
                    z = tmp.tile([P, HID], f32, tag="z")
                    nc.scalar.activation(out=z[:], in_=t1[:], func=AF.Relu)
                    psE = psumT.tile([HID, P], f32, tag="psE")
                    nc.tensor.transpose(out=psE[:], in_=z[:], identity=ident[:])
                    nc.scalar.copy(out=zT_sb[:, t * P:(t + 1) * P], in_=psE[:])
                    # interleave layer-2 A for this tile; fire AG2 quarters
                    tile_A(2, t, None)
                    for q in range(4):
                        if q_last_tile[q] == t:
                            fire_AG(2, q)
                else:
                    mzz = tmp.tile([P, HID], f32, tag="m")
                    nc.vector.tensor_tensor(out=mzz[:], in0=t1[:], in1=Wl_t[:],
                                            op=OP.mult)
                    r = tmp.tile([P, 1], f32, tag="r")
                    nc.vector.tensor_reduce(out=r[:], in_=mzz[:],
                                            axis=mybir.AxisListType.X,
                                            op=OP.add)
                    nc.scalar.activation(out=y_sb[:, t:t + 1], in_=r[:],
                                         func=AF.Sigmoid, bias=bl_t[:, 0:1])

            with tc.tile_pool(name="xt", bufs=1) as xtp:
                xT_t = xtp.tile([P, NLOC], f32, tag="xT")
                nc.sync.dma_start(out=xT_t[:], in_=xT_d[:, :])
                for t in range(NT):
                    tile_A(1, t, xT_t)
                    for q in range(4):
                        if q_last_tile[q] == t:
                            fire_AG(1, q)

            with tc.tile_pool(name="gath", bufs=12) as gp, \
                 tc.tile_pool(name="ohp", bufs=4) as ohp:
                for _w in range(12):
                    gfw = gp.tile([P, BMAX, TABW], bf16, tag="gf")
                    nc.vector.memset(gfw[:], 0.0)
                for t in range(NT):
                    tile_B(1, t, gp, ohp)
                for t in range(NT):
                    tile_B(2, t, gp, ohp)

            psG = psumT.tile([NT, P], f32, tag="psG")
            nc.tensor.matmul(out=psG[:], lhsT=y_sb[:, :NT], rhs=ident[:],
                             start=True, stop=True, is_transpose=True)
            og = tmp.tile([NT, P], f32, tag="og")
            nc.scalar.copy(out=og[:], in_=psG[:])
            nc.sync.dma_start(out=out_d[:, :], in_=og[:])
    nc.compile()
    return nc


def kernel(x, edge_index, W1, b1, W2, b2, Wl, bl):
    from concourse.bass_utils import run_bass_kernel_spmd
    in_maps, meta = _prep(x, edge_index, W1, b1, W2, b2, Wl, bl)
    nc = _program(meta, float(np.asarray(bl).reshape(-1)[0]))
    res = run_bass_kernel_spmd(nc, in_maps, list(range(NCORES)))
    outs = []
    for c in range(NCORES):
        o = np.asarray(res.results[c]["out"], dtype=np.float32).reshape(NLOC)
        outs.append(o[:NLOC_REAL])
    return np.concatenate(outs).reshape(N_NODES, 1).astype(np.float32)


# revision 3
# speedup vs baseline: 1.2655x; 1.0059x over previous
"""Trainium2 Bass kernel v5: v4 + packed gather calls.

Per chunk, cells (tile, chunk) are concatenated at 16-row granularity and
sliced into <=1024-row dma_gather calls (~53 per chunk, ~212 per layer vs 392),
amortizing the ~1us fixed SWDGE descriptor-generation cost per call. A call's
128-slot blocks may span two cells (tiles); each (block, cell) intersection is
a "piece" with its own masked one-hot column, matmul-accumulated into the
owning tile's PSUM.
"""

import numpy as np
import ml_dtypes

P = 128
NCORES = 8
N_NODES = 100000
NLOC_REAL = 12500
NT = 98
NLOC = NT * P
QL = NLOC // 4                 # 3136 local rows per quarter
CR = NCORES * QL               # 25088 rows per quarter table (< 32768)
NCHUNK = 4
IN_C = 128
HID = 64
TABW = 128
CALL_ROWS = 1024
BMAX = CALL_ROWS // P          # 8 blocks per call
OHMAX = 12                     # pieces per call (<= blocks + cells-1)


def _prep(x, edge_index, W1, b1, W2, b2, Wl, bl):
    src = np.asarray(edge_index[0], dtype=np.int64)
    dst = np.asarray(edge_index[1], dtype=np.int64)
    core = dst // NLOC_REAL
    dst_local = dst - core * NLOC_REAL
    sk = src // NLOC_REAL
    si = src - sk * NLOC_REAL
    chunk = si // QL
    src_rel = sk * QL + (si - chunk * QL)
    tile = dst_local // P

    ncell = NT * NCHUNK
    cell = core * ncell + tile * NCHUNK + chunk
    order = np.argsort(cell, kind="stable")
    counts = np.bincount(cell, minlength=NCORES * ncell)
    src_s = src_rel[order].astype(np.int16)
    dstl_s = (dst_local[order] - tile[order] * P).astype(np.float32)
    cell_start = np.zeros(NCORES * ncell + 1, dtype=np.int64)
    np.cumsum(counts, out=cell_start[1:])

    cnt_ctc = counts.reshape(NCORES, NT, NCHUNK)
    r16_tc = np.maximum(16, -(-cnt_ctc.max(axis=0) // 16) * 16)   # [NT, NCHUNK]

    # chunk streams: cell (t, c) occupies stream-c rows [s0[t,c], s0[t,c]+r16)
    s0 = np.zeros((NT, NCHUNK), dtype=np.int64)
    for c in range(NCHUNK):
        s0[1:, c] = np.cumsum(r16_tc[:-1, c])
    stream_len = [int(s0[-1, c] + r16_tc[-1, c]) for c in range(NCHUNK)]

    # calls: slices of each stream, <= CALL_ROWS, aligned to CALL_ROWS grid
    calls = []          # (chunk, s_begin, rows, col_off)
    col_off = 0
    call_of = {}        # (chunk, call_idx_in_chunk) -> global call id
    for c in range(NCHUNK):
        nci = -(-stream_len[c] // CALL_ROWS)
        for j in range(nci):
            a = j * CALL_ROWS
            b = min(stream_len[c], a + CALL_ROWS)
            call_of[(c, j)] = len(calls)
            calls.append((c, a, b - a, col_off))
            col_off += (b - a) // 16
    gcols = col_off

    # pieces: per call, per 128-block, intersections with cells
    # piece: (call_id, blk_in_call, tile, ohcol, lo_in_cell, hi_in_cell, lo_in_blk)
    pieces_by_tile = [[] for _ in range(NT)]
    npieces = 0
    call_pieces = [[] for _ in calls]   # ohcols per call in order
    for cid, (c, a, rows, coff) in enumerate(calls):
        nblk = -(-rows // P)
        for b in range(nblk):
            blo = a + b * P
            bhi = min(a + rows, blo + P)
            # find cells overlapping [blo, bhi) in stream c
            t_lo = int(np.searchsorted(s0[:, c], blo, side="right")) - 1
            t_hi = int(np.searchsorted(s0[:, c], bhi - 1, side="right")) - 1
            for t in range(t_lo, t_hi + 1):
                clo, chi = int(s0[t, c]), int(s0[t, c] + r16_tc[t, c])
                lo = max(blo, clo)
                hi = min(bhi, chi)
                if lo >= hi:
                    continue
                ohcol = npieces
                npieces += 1
                call_pieces[cid].append(ohcol)
                pieces_by_tile[t].append(
                    (cid, b, ohcol, lo - clo, hi - clo, lo - blo))
    assert max(len(cp) for cp in call_pieces) <= OHMAX

    meta = {"calls": calls, "call_pieces": call_pieces,
            "pieces_by_tile": pieces_by_tile, "npieces": npieces,
            "gcols": gcols}

    in_maps = []
    for cc in range(NCORES):
        m = {}
        xl = np.zeros((P, NLOC), dtype=np.float32)
        xl[:, :NLOC_REAL] = np.asarray(
            x[cc * NLOC_REAL:(cc + 1) * NLOC_REAL], np.float32).T
        m["xT"] = np.ascontiguousarray(xl)

        deg = np.bincount(dst_local[core == cc], minlength=NLOC).astype(np.float64)
        deg += 1.0
        m["dis"] = np.ascontiguousarray(
            (1.0 / np.sqrt(deg)).astype(np.float32).reshape(NT, P).T)

        # per-core idx stream per chunk (cells packed, cnt real + pad 0)
        streams = []
        dstv = []        # per-cell dstrel values (cnt real, pad -1 to r16)
        for c in range(NCHUNK):
            s = np.zeros(stream_len[c], dtype=np.int16)
            streams.append(s)
        cellv = {}
        for t in range(NT):
            for c in range(NCHUNK):
                cid2 = cc * ncell + t * NCHUNK + c
                lo, hi = cell_start[cid2], cell_start[cid2 + 1]
                cnt = int(hi - lo)
                r16 = int(r16_tc[t, c])
                a = int(s0[t, c])
                streams[c][a:a + cnt] = src_s[lo:hi]
                dv = np.full(r16, -1.0, dtype=np.float32)
                dv[:cnt] = dstl_s[lo:hi]
                cellv[(t, c)] = dv
        gidx = np.zeros((P, gcols), dtype=np.int16)
        for cid, (c, a, rows, coff) in enumerate(calls):
            seg = streams[c][a:a + rows]
            w = seg.reshape(-1, 16).T
            gidx[:, coff:coff + rows // 16] = np.tile(w, (8, 1))
        m["gidx"] = np.ascontiguousarray(gidx)

        dstrel = np.full((P, npieces), -1.0, dtype=np.float32)
        for t in range(NT):
            for (cid, b, ohcol, lo_c, hi_c, lo_b) in pieces_by_tile[t]:
                c = calls[cid][0]
                dv = cellv[(t, c)]
                n = hi_c - lo_c
                dstrel[lo_b:lo_b + n, ohcol] = dv[lo_c:hi_c]
        m["dstrel"] = np.ascontiguousarray(dstrel.astype(ml_dtypes.bfloat16))

        m["identm"] = np.eye(P, dtype=np.float32)
        m["iota"] = np.ascontiguousarray(
            np.tile(np.arange(P, dtype=np.float32), (P, 1)).astype(
                ml_dtypes.bfloat16))
        m["W1"] = np.asarray(W1, np.float32)
        m["W2"] = np.asarray(W2, np.float32)
        m["b1b"] = np.ascontiguousarray(
            np.tile(np.asarray(b1, np.float32)[None, :], (P, 1)))
        m["b2b"] = np.ascontiguousarray(
            np.tile(np.asarray(b2, np.float32)[None, :], (P, 1)))
        m["Wlb"] = np.ascontiguousarray(
            np.tile(np.asarray(Wl, np.float32)[:, 0][None, :], (P, 1)))
        in_maps.append(m)
    return in_maps, meta


def _program(meta, bl_value, linearize=False):
    from concourse import bass, bacc, mybir
    import concourse.tile as tile

    f32 = mybir.dt.float32
    bf16 = mybir.dt.bfloat16
    i16 = mybir.dt.int16
    AF = mybir.ActivationFunctionType
    OP = mybir.AluOpType

    calls = meta["calls"]
    call_pieces = meta["call_pieces"]
    pieces_by_tile = meta["pieces_by_tile"]
    NPIECES, GCOLS = meta["npieces"], meta["gcols"]

    nc = bacc.Bacc("TRN2", target_bir_lowering=False, debug=False,
                   num_devices=NCORES, num_swdge_queues=4)
    xT_d = nc.dram_tensor("xT", [P, NLOC], f32, kind="ExternalInput")
    dis_d = nc.dram_tensor("dis", [P, NT], f32, kind="ExternalInput")
    gidx_d = nc.dram_tensor("gidx", [P, GCOLS], i16, kind="ExternalInput")
    dstrel_d = nc.dram_tensor("dstrel", [P, NPIECES], bf16, kind="ExternalInput")
    iota_d = nc.dram_tensor("iota", [P, P], bf16, kind="ExternalInput")
    identm_d = nc.dram_tensor("identm", [P, P], f32, kind="ExternalInput")
    W1_d = nc.dram_tensor("W1", [IN_C, HID], f32, kind="ExternalInput")
    W2_d = nc.dram_tensor("W2", [HID, HID], f32, kind="ExternalInput")
    b1b_d = nc.dram_tensor("b1b", [P, HID], f32, kind="ExternalInput")
    b2b_d = nc.dram_tensor("b2b", [P, HID], f32, kind="ExternalInput")
    Wlb_d = nc.dram_tensor("Wlb", [P, HID], f32, kind="ExternalInput")
    out_d = nc.dram_tensor("out", [NT, P], f32, kind="ExternalOutput")

    hlocq = [[nc.dram_tensor(f"h{l}q{q}", [QL, TABW], bf16) for q in range(4)]
             for l in (1, 2)]
    tabq = [[nc.dram_tensor(f"tab{l}q{q}", [CR, TABW], bf16,
                            addr_space="Shared") for q in range(4)]
            for l in (1, 2)]
    rg = [list(range(NCORES))]

    with tile.TileContext(nc, linearize=linearize) as tc:
        from contextlib import ExitStack
        with ExitStack() as ctx:
            const = ctx.enter_context(tc.tile_pool(name="const", bufs=1))
            persist = ctx.enter_context(tc.tile_pool(name="persist", bufs=1))
            tmp = ctx.enter_context(tc.tile_pool(name="tmp", bufs=6))
            psum = ctx.enter_context(tc.tile_pool(name="psum", bufs=3,
                                                  space="PSUM"))
            psumT = ctx.enter_context(tc.tile_pool(name="psumT", bufs=1,
                                                   space="PSUM"))

            ident = const.tile([P, P], f32, tag="ident")
            nc.sync.dma_start(out=ident[:], in_=identm_d[:, :])
            identb = const.tile([P, P], bf16, tag="identb")
            nc.vector.tensor_copy(identb[:], ident[:])
            iota_t = const.tile([P, P], bf16, tag="iota")
            nc.sync.dma_start(out=iota_t[:], in_=iota_d[:, :])
            W1_t = const.tile([IN_C, HID], f32, tag="W1")
            nc.sync.dma_start(out=W1_t[:], in_=W1_d[:, :])
            W2_t = const.tile([HID, HID], f32, tag="W2")
            nc.sync.dma_start(out=W2_t[:], in_=W2_d[:, :])
            b1_t = const.tile([P, HID], f32, tag="b1")
            nc.sync.dma_start(out=b1_t[:], in_=b1b_d[:, :])
            b2_t = const.tile([P, HID], f32, tag="b2")
            nc.sync.dma_start(out=b2_t[:], in_=b2b_d[:, :])
            Wl_t = const.tile([P, HID], f32, tag="Wl")
            nc.sync.dma_start(out=Wl_t[:], in_=Wlb_d[:, :])
            bl_t = const.tile([P, 1], f32, tag="bl")
            nc.vector.memset(bl_t[:], float(bl_value))
            dis_t = const.tile([P, NT], f32, tag="dis")
            nc.sync.dma_start(out=dis_t[:], in_=dis_d[:, :])
            dstrel_t = const.tile([P, NPIECES], bf16, tag="dstrel")
            nc.sync.dma_start(out=dstrel_t[:], in_=dstrel_d[:, :])
            gidx_t = const.tile([P, GCOLS], i16, tag="gidx")
            nc.sync.dma_start(out=gidx_t[:], in_=gidx_d[:, :])

            hp_sb1 = persist.tile([P, NT * HID], bf16, tag="hp_sb1")
            hp_sb2 = persist.tile([P, NT * HID], bf16, tag="hp_sb2")
            hp_sb = [hp_sb1, hp_sb2]
            zT_sb = persist.tile([HID, NT * P], f32, tag="zT_sb")
            y_sb = persist.tile([P, NT], f32, tag="y_sb")

            nreg = {}

            def reg_for(n):
                if n not in nreg:
                    nreg[n] = nc.gpsimd.to_reg(n)
                return nreg[n]

            def tile_A(l, t, xT_t):
                W_t = W1_t if l == 1 else W2_t
                ps = psum.tile([P, HID], f32, tag="psA")
                if l == 1:
                    lhsT = xT_t[:, t * P:(t + 1) * P]
                else:
                    lhsT = zT_sb[:, t * P:(t + 1) * P]
                nc.tensor.matmul(out=ps[:], lhsT=lhsT, rhs=W_t[:],
                                 start=True, stop=True)
                hp = hp_sb[l - 1][:, t * HID:(t + 1) * HID]
                nc.scalar.activation(out=hp, in_=ps[:], func=AF.Copy,
                                     scale=dis_t[:, t:t + 1])
                r0, r1 = t * P, (t + 1) * P
                q0, q1 = r0 // QL, (r1 - 1) // QL
                for q in range(q0, q1 + 1):
                    a = max(r0, q * QL)
                    b = min(r1, (q + 1) * QL)
                    nc.sync.dma_start(
                        out=hlocq[l - 1][q][a - q * QL:b - q * QL, 0:HID],
                        in_=hp[a - r0:b - r0, :])

            def fire_AG(l, q):
                nc.gpsimd.collective_compute(
                    "AllGather", mybir.AluOpType.bypass, replica_groups=rg,
                    ins=[hlocq[l - 1][q][:, :]], outs=[tabq[l - 1][q][:, :]])

            q_last_tile = [-(-(q + 1) * QL // P) - 1 for q in range(4)]
            qn_state = [0]

            def layer_B(l, gp, ohp, interleave_A2):
                """emit calls lazily in tile order; piece-matmuls per tile."""
                b_t = b1_t if l == 1 else b2_t
                gf_of = {}
                oh_of = {}

                def emit_call(cid):
                    (c, a, rows, coff) = calls[cid]
                    nblk = -(-rows // P)
                    # one-hot for all pieces of this call
                    pcs = call_pieces[cid]
                    oh = ohp.tile([P, OHMAX, P], bf16, tag="oh")
                    npc = len(pcs)
                    # pieces are consecutive ohcols by construction
                    oc0 = pcs[0]
                    assert pcs == list(range(oc0, oc0 + npc))
                    nc.vector.tensor_tensor(
                        out=oh[:, 0:npc, :],
                        in0=iota_t[:].unsqueeze(1).broadcast_to([P, npc, P]),
                        in1=dstrel_t[:, oc0:oc0 + npc]
                            .unsqueeze(2).broadcast_to([P, npc, P]),
                        op=OP.is_equal)
                    gf = gp.tile([P, BMAX, TABW], bf16, tag="gf")
                    nc.gpsimd.dma_gather(
                        out_ap=gf[:, 0:nblk, :],
                        in_ap=tabq[l - 1][c][:, :],
                        idxs_ap=gidx_t[:, coff:coff + rows // 16],
                        num_idxs=rows, num_idxs_reg=reg_for(rows),
                        elem_size=TABW, queue_num=qn_state[0])
                    qn_state[0] = (qn_state[0] + 1) % 4
                    gf_of[cid] = gf
                    oh_of[cid] = (oh, oc0)

                for t in range(NT):
                    for (cid, b, ohcol, lo_c, hi_c, lo_b) in pieces_by_tile[t]:
                        if cid not in gf_of:
                            emit_call(cid)
                    ps = psum.tile([P, HID], f32, tag="psB")
                    nc.tensor.matmul(
                        out=ps[:], lhsT=identb[:],
                        rhs=hp_sb[l - 1][:, t * HID:(t + 1) * HID],
                        start=True, stop=False)
                    npieces_t = len(pieces_by_tile[t])
                    for i, (cid, b, ohcol, lo_c, hi_c, lo_b) in enumerate(
                            pieces_by_tile[t]):
                        oh, oc0 = oh_of[cid]
                        nc.tensor.matmul(
                            out=ps[:],
                            lhsT=oh[:, ohcol - oc0, :],
                            rhs=gf_of[cid][:, b, 0:HID],
                            start=False,
                            stop=(i == npieces_t - 1))
                    t1 = tmp.tile([P, HID], f32, tag="t1")
                    nc.scalar.activation(out=t1[:], in_=ps[:], func=AF.Copy,
                                         scale=dis_t[:, t:t + 1])
                    nc.vector.tensor_tensor(out=t1[:], in0=t1[:], in1=b_t[:],
                                            op=OP.add)
                    if l == 1:
                        z = tmp.tile([P, HID], f32, tag="z")
                        nc.scalar.activation(out=z[:], in_=t1[:], func=AF.Relu)
                        psE = psumT.tile([HID, P], f32, tag="psE")
                        nc.tensor.transpose(out=psE[:], in_=z[:],
                                            identity=ident[:])
                        nc.scalar.copy(out=zT_sb[:, t * P:(t + 1) * P],
                                       in_=psE[:])
                        if interleave_A2:
                            tile_A(2, t, None)
                            for q in range(4):
                                if q_last_tile[q] == t:
                                    fire_AG(2, q)
                    else:
                        mzz = tmp.tile([P, HID], f32, tag="m")
                        nc.vector.tensor_tensor(out=mzz[:], in0=t1[:],
                                                in1=Wl_t[:], op=OP.mult)
                        r = tmp.tile([P, 1], f32, tag="r")
                        nc.vector.tensor_reduce(out=r[:], in_=mzz[:],
                                                axis=mybir.AxisListType.X,
                                                op=OP.add)
                        nc.scalar.activation(out=y_sb[:, t:t + 1], in_=r[:],
                                             func=AF.Sigmoid, bias=bl_t[:, 0:1])

            with tc.tile_pool(name="xt", bufs=1) as xtp:
                xT_t = xtp.tile([P, NLOC], f32, tag="xT")
                nc.sync.dma_start(out=xT_t[:], in_=xT_d[:, :])
                for t in range(NT):
                    tile_A(1, t, xT_t)
                    for q in range(4):
                        if q_last_tile[q] == t:
                            fire_AG(1, q)

            with tc.tile_pool(name="gath", bufs=26) as gp, \
                 tc.tile_pool(name="ohp", bufs=10) as ohp:
                for _w in range(26):
                    gfw = gp.tile([P, BMAX, TABW], bf16, tag="gf")
                    nc.vector.memset(gfw[:], 0.0)
                layer_B(1, gp, ohp, True)
                layer_B(2, gp, ohp, False)

            psG = psumT.tile([NT, P], f32, tag="psG")
            nc.tensor.matmul(out=psG[:], lhsT=y_sb[:, :NT], rhs=ident[:],
                             start=True, stop=True, is_transpose=True)
            og = tmp.tile([NT, P], f32, tag="og")
            nc.scalar.copy(out=og[:], in_=psG[:])
            nc.sync.dma_start(out=out_d[:, :], in_=og[:])
    nc.compile()
    return nc


def kernel(x, edge_index, W1, b1, W2, b2, Wl, bl):
    from concourse.bass_utils import run_bass_kernel_spmd
    in_maps, meta = _prep(x, edge_index, W1, b1, W2, b2, Wl, bl)
    nc = _program(meta, float(np.asarray(bl).reshape(-1)[0]))
    res = run_bass_kernel_spmd(nc, in_maps, list(range(NCORES)))
    outs = []
    for c in range(NCORES):
        o = np.asarray(res.results[c]["out"], dtype=np.float32).reshape(NLOC)
        outs.append(o[:NLOC_REAL])
    return np.concatenate(outs).reshape(N_NODES, 1).astype(np.float32)
